# revision 16
# baseline (speedup 1.0000x reference)
"""Trainium2 Bass kernel for a contrastive hinge loss (bf16 rewrite).

Problem (B=32 splits, L=1024 candidates/split, P=8 positives/split, D=256):
    e = l2norm(sent), q = l2norm(query)
    sim[b,l] = e[b,l] . q[b]
    loss = sum_{b, p in pos_b, j in neg_b} relu(sim[b,j] - sim[b,p] + margin) / total

Strategy (data-parallel over B across 8 cores, 4 splits per core):
  Layout: D on partitions (2 chunks of 128), candidates on the free dim,
  everything bf16 on the wire (half the HBM traffic of fp32; PE runs
  1 col/cycle instead of fp32's multi-pass).

  Device per core:
    - x2 [128, 8*1024] bf16: tile t = (split k, d-chunk c) at cols t*1024.
      4 DMA chunks of 512KB, two HWDGE rings.
    - dot[k,l]: PE matmuls, lhsT = one-hot column matrix (col k = qhat_k
      chunk c), accumulating [4, 1024] in PSUM @p0-3 (col-group 0).
    - ssq[k,l]: squares on DVE/ACT/GpSimd, then PE matmuls with one-hot
      ones lhsT into PSUM @p32-35 (col-group 1, runs concurrent with dot).
    - sim = dot * rsqrt(ssq): ACT Sqrt (PSUM->SBUF) -> DVE reciprocal ->
      DVE mult (PSUM fp32 x SBUF fp32 -> bf16, cross-partition-base OK for
      mixed-space operands).
    - positives: host gathers the 32 positive columns; tiny PE matmuls
      (q.xP and Gram(xP)) @p64-95 + diagonal-mask STT give s_vec[32],
      ms = margin - s_vec.
    - hinge: replicate sim rows to 32 partitions via PE (lhsT = selector E),
      then ONE ACT Relu pass per column half with per-partition bias = ms
      and accum_out -> G[32].
  Host: normalizes queries, builds one-hot weights, gathers positives;
  finish: loss = [sum G - sum_{p,q in pos} relu(s_q - s_p + m)] / total.
"""

import numpy as np
import ml_dtypes

B, L, P, D = 32, 1024, 8, 256
NCORES = 8
BL = B // NCORES          # 4 splits per core
T = BL * 2                # 8 (split, chunk) tiles per core
MARGIN = 0.01
NWARM = 5                 # PE warm-up matmuls (HAM clock-gate)

# aux column layout (all bf16, [128, AUXC])
QWS_O = 0                 # 2c x 4k blocks of [128,4]: col k = qhat chunk c
OWS_O = 32                # 4k blocks of [128,4]: col k = ones
EP_O = 48                 # [4, 32] selector E[b, m] = (m//8 == b)
QP_O = 80                 # [128, 2*32]: col (c,k*8+j) = qhat_k chunk c
XP_O = 144                # [128, 2*32]: col (c,k*8+j) = x[k, pos_kj] chunk c
I32_O = 208               # [32,32] identity at partitions 64..95
AUXC = 240

# which engine squares tile t = k*2+c  (v=DVE, s=ACT, g=GpSimd)
SQ_ENG = ["v", "g", "s", "g", "v", "s", "v", "v"]

_CACHED = {}


def _build_nc():
    import concourse.bass as bass
    import concourse.mybir as mybir
    import concourse.tile as tile
    from concourse import bacc

    f32 = mybir.dt.float32
    bf16 = mybir.dt.bfloat16
    Alu = mybir.AluOpType
    Act = mybir.ActivationFunctionType

    nc = bacc.Bacc("TRN2")
    x2 = nc.dram_tensor("x2", [128, T * 1024], bf16, kind="ExternalInput")
    aux = nc.dram_tensor("aux", [128, AUXC], bf16, kind="ExternalInput")
    outp = nc.dram_tensor("outp", [32, 3], f32, kind="ExternalOutput")

    with tile.TileContext(nc) as tc:
        with (
            tc.tile_pool(name="sing", bufs=1) as sing,
            tc.tile_pool(name="pp", bufs=1, space="PSUM") as pp,
        ):
            aux_sb = sing.tile([128, AUXC], bf16, name="aux_sb")
            x_sb = sing.tile([128, T * 1024], bf16, name="x_sb")
            # aux first on the SP ring so the pos stage + qws unblock early
            nc.sync.dma_start(out=aux_sb[:, :], in_=aux[:, :])
            # x in 8 tiles of 1024 cols over three DMA queues (each HWDGE
            # ring saturates at ~150 GB/s; SP + ACT + SWDGE together reach
            # the HBM limit)
            DMA_ENG = [nc.scalar, nc.sync, nc.gpsimd]
            for t in range(T):
                DMA_ENG[t % 3].dma_start(
                    out=x_sb[:, t * 1024:(t + 1) * 1024],
                    in_=x2[:, t * 1024:(t + 1) * 1024])

            # PSUM layout: one accumulation group per 2KB bank -- a start=True
            # matmul into a bank wipes any open accumulation there (verified
            # on HW), so dot/ssq/pos/rep each get private banks.
            # dot-h0 @[0:4] bank0 (col-grp 0), dot-h1 @[64:68] bank1 (grp 2),
            # ssq-h0 @[32:36] bank2 (grp 1), ssq-h1 @[96:100] bank3 (grp 3):
            # four concurrent rhs streams through the PE.
            psMain = pp.tile([128, 2048], f32, name="psMain")  # banks 0-3
            psRep = pp.tile([128, 1024], f32, name="psRep")    # banks 4-5
            psPos = pp.tile([96, 1024], f32, name="psPos")     # banks 6-7

            # ---- warmups (M=128 so the HAM activity monitor sees them) ----
            warm_sb = sing.tile([128, 512], bf16, name="warm_sb")
            nc.vector.memset(warm_sb[:, :], 0.0)
            for i in range(NWARM):
                nc.tensor.matmul(
                    psRep[0:128, 0:512], lhsT=warm_sb[:, 0:128],
                    rhs=warm_sb[:, :],
                    start=True, stop=True, skip_group_check=True)

            # ---- positives (tiny, early; only needs aux) ----
            for c in range(2):
                qp = aux_sb[:, QP_O + c * 32:QP_O + (c + 1) * 32]
                xp = aux_sb[:, XP_O + c * 32:XP_O + (c + 1) * 32]
                nc.tensor.matmul(
                    psPos[64:96, 0:32], lhsT=qp, rhs=xp,
                    start=(c == 0), stop=(c == 1), skip_group_check=True)
                nc.tensor.matmul(
                    psPos[64:96, 512:544], lhsT=xp, rhs=xp,
                    start=(c == 0), stop=(c == 1), skip_group_check=True)
            pos_sb = sing.tile([96, 8], f32, name="pos_sb")
            junkP = sing.tile([96, 64], f32, name="junkP")
            i32_sb = aux_sb[64:96, I32_O:I32_O + 32]
            nc.vector.scalar_tensor_tensor(
                out=junkP[64:96, 0:32], in0=psPos[64:96, 0:32], scalar=1.0,
                in1=i32_sb, op0=Alu.mult, op1=Alu.mult,
                accum_out=pos_sb[64:96, 0:1])
            nc.vector.scalar_tensor_tensor(
                out=junkP[64:96, 32:64], in0=psPos[64:96, 512:544], scalar=1.0,
                in1=i32_sb, op0=Alu.mult, op1=Alu.mult,
                accum_out=pos_sb[64:96, 1:2])
            nc.scalar.activation(
                out=pos_sb[64:96, 2:3], in_=pos_sb[64:96, 1:2], func=Act.Sqrt)
            nc.vector.reciprocal(
                out=pos_sb[64:96, 3:4], in_=pos_sb[64:96, 2:3])
            nc.vector.tensor_mul(
                out=pos_sb[64:96, 4:5], in0=pos_sb[64:96, 0:1],
                in1=pos_sb[64:96, 3:4])
            out_sb = sing.tile([128, 3], f32, name="out_sb")
            ms_sb = sing.tile([128, 1], f32, name="ms_sb")
            # ms = margin - s_vec, moved to partitions 96..127 for the G pass
            nc.scalar.activation(
                out=ms_sb[96:128, 0:1], in_=pos_sb[64:96, 4:5],
                func=Act.Copy, bias=float(MARGIN), scale=-1.0)
            # s_vec to the output block (also cross-partition copy)
            nc.scalar.activation(
                out=out_sb[96:128, 2:3], in_=pos_sb[64:96, 4:5], func=Act.Copy)

            # ---- main: squares + dot/ssq matmuls per (split k, chunk c) ----
            # dot/ssq x h0/h1 go to four distinct PE column-groups, so the
            # four streams execute concurrently on the array.
            DOT_PS = [(slice(0, 4), slice(0, 512), (0, 0)),
                      (slice(64, 68), slice(512, 1024), (0, 64))]
            SSQ_PS = [(slice(32, 36), slice(1024, 1536), (0, 32)),
                      (slice(96, 100), slice(1536, 2048), (0, 96))]
            xsq_sb = sing.tile([128, T * 1024], bf16, name="xsq_sb")
            for k in range(BL):
                for c in range(2):
                    t = k * 2 + c
                    seg = slice(t * 1024, (t + 1) * 1024)
                    e = SQ_ENG[t]
                    if e == "s":
                        nc.scalar.activation(
                            out=xsq_sb[:, seg], in_=x_sb[:, seg],
                            func=Act.Square)
                    elif e == "v":
                        nc.vector.tensor_mul(
                            out=xsq_sb[:, seg], in0=x_sb[:, seg],
                            in1=x_sb[:, seg])
                    else:
                        nc.gpsimd.tensor_mul(
                            out=xsq_sb[:, seg], in0=x_sb[:, seg],
                            in1=x_sb[:, seg])
                    qw = aux_sb[:, QWS_O + (c * 4 + k) * 4:
                                QWS_O + (c * 4 + k) * 4 + 4]
                    ow = aux_sb[:, OWS_O + k * 4:OWS_O + k * 4 + 4]
                    first = (k == 0 and c == 0)
                    last = (k == BL - 1 and c == 1)
                    horder = (1, 0) if last else (0, 1)
                    for h in horder:
                        xs = x_sb[:, t * 1024 + h * 512:t * 1024 + h * 512 + 512]
                        xq = xsq_sb[:, t * 1024 + h * 512:t * 1024 + h * 512 + 512]
                        dp, dc, dtp = DOT_PS[h]
                        sp, sc, stp = SSQ_PS[h]
                        nc.tensor.matmul(
                            psMain[dp, dc], lhsT=qw, rhs=xs,
                            start=first, stop=last, skip_group_check=True,
                            tile_position=dtp)
                        nc.tensor.matmul(
                            psMain[sp, sc], lhsT=ow, rhs=xq,
                            start=first, stop=last, skip_group_check=True,
                            tile_position=stp)

            # ---- finish: sim, replicate, hinge-accumulate ----
            # reciprocal_approx_fast requires partition base 0 (custom-DVE
            # uop breaks on sliced bases) -- keep the whole chain at p0-3
            sq_s = sing.tile([4, 1024], f32, name="sq_s")
            r_s = sing.tile([4, 1024], f32, name="r_s")
            sim_sb = sing.tile([4, 1024], bf16, name="sim_sb")
            junkG = sing.tile([128, 1024], bf16, name="junkG")
            ep_sb = aux_sb[0:4, EP_O:EP_O + 32]
            # h1 finishes first (last tile emits its h1 matmuls first), so
            # drive the whole h1 chain ahead of h0 on every engine
            for h in (1, 0):
                hs = slice(h * 512, (h + 1) * 512)
                sp, sc, _ = SSQ_PS[h]
                nc.scalar.activation(
                    out=sq_s[0:4, hs], in_=psMain[sp, sc], func=Act.Sqrt)
            for h in (1, 0):
                hs = slice(h * 512, (h + 1) * 512)
                dp, dc, _ = DOT_PS[h]
                nc.vector.reciprocal_approx_fast(
                    out=r_s[0:4, hs], in_=sq_s[0:4, hs])
                nc.vector.tensor_mul(
                    out=sim_sb[:, hs], in0=psMain[dp, dc], in1=r_s[0:4, hs])
            for h in (1, 0):
                hs = slice(h * 512, (h + 1) * 512)
                nc.tensor.matmul(
                    psRep[96:128, hs], lhsT=ep_sb, rhs=sim_sb[:, hs],
                    start=True, stop=True, skip_group_check=True,
                    tile_position=(0, 96))
            # hinge accumulate: h1 on ACT (relu+bias), h0 on DVE
            # ((simrep + ms) max 0, accumulated) -- the two run in parallel
            nc.scalar.activation(
                out=junkG[96:128, 512:1024], in_=psRep[96:128, 512:1024],
                func=Act.Relu, bias=ms_sb[96:128, 0:1], scale=1.0,
                accum_out=out_sb[96:128, 1:2])
            nc.vector.scalar_tensor_tensor(
                out=junkG[96:128, 0:512],
                in0=psRep[96:128, 0:512], scalar=ms_sb[96:128, 0:1],
                in1=warm_sb[96:128, 0:512],
                op0=Alu.add, op1=Alu.max,
                accum_out=out_sb[96:128, 0:1])

            nc.sync.dma_start(out=outp[:, :], in_=out_sb[96:128, 0:3])

    nc.finalize()
    return nc


def _get_nc():
    if "nc" not in _CACHED:
        _CACHED["nc"] = _build_nc()
    return _CACHED["nc"]


def _host_prep(sent, query, pos_idx):
    """Build per-core input maps (all heavy prep is reshapes + bf16 cast)."""
    bf16 = ml_dtypes.bfloat16
    sent = np.ascontiguousarray(sent, dtype=np.float32)
    query = np.asarray(query, dtype=np.float32)
    pos_idx = np.asarray(pos_idx).astype(np.int64)

    qn = np.linalg.norm(query, axis=-1, keepdims=True)
    qhat = (query / np.maximum(qn, 1e-12)).astype(bf16)     # [B, D]

    # [B, 2, 128, L] bf16, d-chunk-major transposed tiles
    xt = sent.astype(bf16).transpose(0, 2, 1).reshape(B, 2, 128, L)

    in_maps = []
    for core in range(NCORES):
        ks = slice(core * BL, (core + 1) * BL)
        x2 = np.ascontiguousarray(
            xt[ks].transpose(2, 0, 1, 3).reshape(128, T * 1024))

        aux = np.zeros((128, AUXC), dtype=bf16)
        for c in range(2):
            for k in range(BL):
                aux[:, QWS_O + (c * 4 + k) * 4 + k] = qhat[core * BL + k,
                                                           c * 128:(c + 1) * 128]
        for k in range(BL):
            aux[:, OWS_O + k * 4 + k] = 1.0
        for k in range(BL):
            aux[k, EP_O + k * P:EP_O + (k + 1) * P] = 1.0
        for c in range(2):
            for k in range(BL):
                for j in range(P):
                    aux[:, QP_O + c * 32 + k * P + j] = qhat[
                        core * BL + k, c * 128:(c + 1) * 128]
                    aux[:, XP_O + c * 32 + k * P + j] = xt[
                        core * BL + k, c, :, pos_idx[core * BL + k, j]]
        aux[np.arange(64, 96), I32_O + np.arange(32)] = 1.0

        in_maps.append({"x2": x2, "aux": aux})
    return in_maps, pos_idx


def _host_finish(results, pos_idx):
    """Combine per-core (G[k,j], s_vec[k,j]) into the scalar loss."""
    g = np.zeros((B, P), dtype=np.float64)
    s = np.zeros((B, P), dtype=np.float64)
    for core, res in enumerate(results):
        o = res["outp"].astype(np.float64)          # [32, 3]
        g[core * BL:(core + 1) * BL] = (o[:, 0] + o[:, 1]).reshape(BL, P)
        s[core * BL:(core + 1) * BL] = o[:, 2].reshape(BL, P)

    loss = 0.0
    total = 0
    for b in range(B):
        _, first = np.unique(pos_idx[b], return_index=True)
        npos = len(first)
        total += npos * (L - npos)
        sb = s[b, first]
        loss += g[b, first].sum()
        loss -= np.maximum(sb[None, :] - sb[:, None] + MARGIN, 0.0).sum()
    return np.float32(loss / total)


def kernel(sent_embeddings, query_embeddings, pos_idx, splits=None, **_):
    import sys
    if "/opt/trn_rl_repo" not in sys.path:
        sys.path.insert(0, "/opt/trn_rl_repo")
    from concourse.bass_utils import run_bass_kernel_spmd

    in_maps, pos_idx = _host_prep(sent_embeddings, query_embeddings, pos_idx)
    nc = _get_nc()
    res = run_bass_kernel_spmd(nc, in_maps, core_ids=list(range(NCORES)))
    _CACHED["last_result"] = res
    return _host_finish(res.results, pos_idx)


if __name__ == "__main__":
    rng = np.random.default_rng(0)
    sent = rng.standard_normal((B, L, D), dtype=np.float32)
    query = rng.standard_normal((B, D), dtype=np.float32)
    pidx = np.stack([rng.choice(L, P, replace=False) for _ in range(B)])
    print(kernel(sent, query, pidx, L))


# revision 17
# speedup vs baseline: 1.0193x; 1.0193x over previous
"""Trainium2 Bass kernel for a contrastive hinge loss (bf16 rewrite).

Problem (B=32 splits, L=1024 candidates/split, P=8 positives/split, D=256):
    e = l2norm(sent), q = l2norm(query)
    sim[b,l] = e[b,l] . q[b]
    loss = sum_{b, p in pos_b, j in neg_b} relu(sim[b,j] - sim[b,p] + margin) / total

Strategy (data-parallel over B across 8 cores, 4 splits per core):
  Layout: D on partitions (2 chunks of 128), candidates on the free dim,
  everything bf16 on the wire (half the HBM traffic of fp32; PE runs
  1 col/cycle instead of fp32's multi-pass).

  Device per core:
    - x2 [128, 8*1024] bf16: tile t = (split k, d-chunk c) at cols t*1024.
      4 DMA chunks of 512KB, two HWDGE rings.
    - dot[k,l]: PE matmuls, lhsT = one-hot column matrix (col k = qhat_k
      chunk c), accumulating [4, 1024] in PSUM @p0-3 (col-group 0).
    - ssq[k,l]: squares on DVE/ACT/GpSimd, then PE matmuls with one-hot
      ones lhsT into PSUM @p32-35 (col-group 1, runs concurrent with dot).
    - sim = dot * rsqrt(ssq): ACT Sqrt (PSUM->SBUF) -> DVE reciprocal ->
      DVE mult (PSUM fp32 x SBUF fp32 -> bf16, cross-partition-base OK for
      mixed-space operands).
    - positives: host gathers the 32 positive columns; tiny PE matmuls
      (q.xP and Gram(xP)) @p64-95 + diagonal-mask STT give s_vec[32],
      ms = margin - s_vec.
    - hinge: replicate sim rows to 32 partitions via PE (lhsT = selector E),
      then ONE ACT Relu pass per column half with per-partition bias = ms
      and accum_out -> G[32].
  Host: normalizes queries, builds one-hot weights, gathers positives;
  finish: loss = [sum G - sum_{p,q in pos} relu(s_q - s_p + m)] / total.
"""

import numpy as np
import ml_dtypes

B, L, P, D = 32, 1024, 8, 256
NCORES = 8
BL = B // NCORES          # 4 splits per core
T = BL * 2                # 8 (split, chunk) tiles per core
MARGIN = 0.01
NWARM = 5                 # PE warm-up matmuls (HAM clock-gate)

# aux column layout (all bf16, [128, AUXC])
QWS_O = 0                 # 2c x 4k blocks of [128,4]: col k = qhat chunk c
OWS_O = 32                # 4k blocks of [128,4]: col k = ones
EP_O = 48                 # [4, 32] selector E[b, m] = (m//8 == b)
QP_O = 80                 # [128, 2*32]: col (c,k*8+j) = qhat_k chunk c
XP_O = 144                # [128, 2*32]: col (c,k*8+j) = x[k, pos_kj] chunk c
I32_O = 208               # [32,32] identity at partitions 64..95
AUXC = 240

# which engine squares tile t = k*2+c  (v=DVE, s=ACT, g=GpSimd)
SQ_ENG = ["g", "g", "s", "v", "s", "g", "v", "v"]

_CACHED = {}


def _build_nc():
    import concourse.bass as bass
    import concourse.mybir as mybir
    import concourse.tile as tile
    from concourse import bacc

    f32 = mybir.dt.float32
    bf16 = mybir.dt.bfloat16
    Alu = mybir.AluOpType
    Act = mybir.ActivationFunctionType

    nc = bacc.Bacc("TRN2")
    x2 = nc.dram_tensor("x2", [128, T * 1024], bf16, kind="ExternalInput")
    aux = nc.dram_tensor("aux", [128, AUXC], bf16, kind="ExternalInput")
    outp = nc.dram_tensor("outp", [32, 3], f32, kind="ExternalOutput")

    with tile.TileContext(nc) as tc:
        with (
            tc.tile_pool(name="sing", bufs=1) as sing,
            tc.tile_pool(name="pp", bufs=1, space="PSUM") as pp,
        ):
            aux_sb = sing.tile([128, AUXC], bf16, name="aux_sb")
            x_sb = sing.tile([128, T * 1024], bf16, name="x_sb")
            # aux first on the SP ring so the pos stage + qws unblock early
            nc.sync.dma_start(out=aux_sb[:, :], in_=aux[:, :])
            # x in 8 tiles of 1024 cols over three DMA queues (each HWDGE
            # ring saturates at ~150 GB/s; SP + ACT + SWDGE together reach
            # the HBM limit)
            DMA_ENG = [nc.scalar, nc.sync]
            for t in range(T):
                DMA_ENG[t % 2].dma_start(
                    out=x_sb[:, t * 1024:(t + 1) * 1024],
                    in_=x2[:, t * 1024:(t + 1) * 1024])

            # PSUM layout: one accumulation group per 2KB bank -- a start=True
            # matmul into a bank wipes any open accumulation there (verified
            # on HW), so dot/ssq/pos/rep each get private banks.
            # dot-h0 @[0:4] bank0 (col-grp 0), dot-h1 @[64:68] bank1 (grp 2),
            # ssq-h0 @[32:36] bank2 (grp 1), ssq-h1 @[96:100] bank3 (grp 3):
            # four concurrent rhs streams through the PE.
            # separate pp.tile per bank: Tile tracks PSUM deps per tile,
            # so readers of one region must not be chained to writers of
            # another (e.g. sqrt-h1 must not wait on the h0 matmuls)
            psDot0 = pp.tile([4, 512], f32, name="psDot0")
            psSsq0 = pp.tile([36, 512], f32, name="psSsq0")
            psDot1 = pp.tile([68, 512], f32, name="psDot1")
            psSsq1 = pp.tile([100, 512], f32, name="psSsq1")
            psRep0 = pp.tile([128, 512], f32, name="psRep0")
            psRep1 = pp.tile([128, 512], f32, name="psRep1")
            psPosA = pp.tile([96, 512], f32, name="psPosA")
            psPosB = pp.tile([96, 512], f32, name="psPosB")

            # ---- warmups (M=128 so the HAM activity monitor sees them) ----
            warm_sb = sing.tile([128, 512], bf16, name="warm_sb")
            nc.vector.memset(warm_sb[:, :], 0.0)
            for i in range(NWARM):
                nc.tensor.matmul(
                    psRep0[0:128, 0:512], lhsT=warm_sb[:, 0:128],
                    rhs=warm_sb[:, :],
                    start=True, stop=True, skip_group_check=True)

            # ---- positives (tiny, early; only needs aux) ----
            for c in range(2):
                qp = aux_sb[:, QP_O + c * 32:QP_O + (c + 1) * 32]
                xp = aux_sb[:, XP_O + c * 32:XP_O + (c + 1) * 32]
                nc.tensor.matmul(
                    psPosA[64:96, 0:32], lhsT=qp, rhs=xp,
                    start=(c == 0), stop=(c == 1), skip_group_check=True)
                nc.tensor.matmul(
                    psPosB[64:96, 0:32], lhsT=xp, rhs=xp,
                    start=(c == 0), stop=(c == 1), skip_group_check=True)
            pos_sb = sing.tile([96, 8], f32, name="pos_sb")
            junkP = sing.tile([96, 64], f32, name="junkP")
            i32_sb = aux_sb[64:96, I32_O:I32_O + 32]
            nc.vector.scalar_tensor_tensor(
                out=junkP[64:96, 0:32], in0=psPosA[64:96, 0:32], scalar=1.0,
                in1=i32_sb, op0=Alu.mult, op1=Alu.mult,
                accum_out=pos_sb[64:96, 0:1])
            nc.vector.scalar_tensor_tensor(
                out=junkP[64:96, 32:64], in0=psPosB[64:96, 0:32], scalar=1.0,
                in1=i32_sb, op0=Alu.mult, op1=Alu.mult,
                accum_out=pos_sb[64:96, 1:2])
            nc.scalar.activation(
                out=pos_sb[64:96, 2:3], in_=pos_sb[64:96, 1:2], func=Act.Sqrt)
            nc.vector.reciprocal(
                out=pos_sb[64:96, 3:4], in_=pos_sb[64:96, 2:3])
            nc.vector.tensor_mul(
                out=pos_sb[64:96, 4:5], in0=pos_sb[64:96, 0:1],
                in1=pos_sb[64:96, 3:4])
            out_sb = sing.tile([128, 3], f32, name="out_sb")
            ms_sb = sing.tile([128, 1], f32, name="ms_sb")
            # ms = margin - s_vec, moved to partitions 96..127 for the G pass
            nc.scalar.activation(
                out=ms_sb[96:128, 0:1], in_=pos_sb[64:96, 4:5],
                func=Act.Copy, bias=float(MARGIN), scale=-1.0)
            # s_vec to the output block (also cross-partition copy)
            nc.scalar.activation(
                out=out_sb[96:128, 2:3], in_=pos_sb[64:96, 4:5], func=Act.Copy)

            # ---- main: squares + dot/ssq matmuls per (split k, chunk c) ----
            # dot/ssq x h0/h1 go to four distinct PE column-groups, so the
            # four streams execute concurrently on the array.
            DOT_T = [(psDot0, slice(0, 4), (0, 0)),
                     (psDot1, slice(64, 68), (0, 64))]
            SSQ_T = [(psSsq0, slice(32, 36), (0, 32)),
                     (psSsq1, slice(96, 100), (0, 96))]
            xsq_sb = sing.tile([128, T * 1024], bf16, name="xsq_sb")
            for k in range(BL):
                for c in range(2):
                    t = k * 2 + c
                    seg = slice(t * 1024, (t + 1) * 1024)
                    e = SQ_ENG[t]
                    if e == "s":
                        nc.scalar.activation(
                            out=xsq_sb[:, seg], in_=x_sb[:, seg],
                            func=Act.Square)
                    elif e == "v":
                        nc.vector.tensor_mul(
                            out=xsq_sb[:, seg], in0=x_sb[:, seg],
                            in1=x_sb[:, seg])
                    else:
                        nc.gpsimd.tensor_mul(
                            out=xsq_sb[:, seg], in0=x_sb[:, seg],
                            in1=x_sb[:, seg])
                    qw = aux_sb[:, QWS_O + (c * 4 + k) * 4:
                                QWS_O + (c * 4 + k) * 4 + 4]
                    ow = aux_sb[:, OWS_O + k * 4:OWS_O + k * 4 + 4]
                    first = (k == 0 and c == 0)
                    last = (k == BL - 1 and c == 1)
                    horder = (1, 0) if last else (0, 1)
                    for h in horder:
                        xs = x_sb[:, t * 1024 + h * 512:t * 1024 + h * 512 + 512]
                        xq = xsq_sb[:, t * 1024 + h * 512:t * 1024 + h * 512 + 512]
                        dt_, dp, dtp = DOT_T[h]
                        st_, sp, stp = SSQ_T[h]
                        nc.tensor.matmul(
                            dt_[dp, :], lhsT=qw, rhs=xs,
                            start=first, stop=last, skip_group_check=True,
                            tile_position=dtp)
                        nc.tensor.matmul(
                            st_[sp, :], lhsT=ow, rhs=xq,
                            start=first, stop=last, skip_group_check=True,
                            tile_position=stp)

            # ---- finish: sim, replicate, hinge-accumulate ----
            # reciprocal_approx_fast requires partition base 0 (custom-DVE
            # uop breaks on sliced bases) -- keep the whole chain at p0-3
            sq_s = sing.tile([4, 1024], f32, name="sq_s")
            r_s = sing.tile([4, 1024], f32, name="r_s")
            sim_sb = sing.tile([4, 1024], bf16, name="sim_sb")
            junkG = sing.tile([128, 1024], bf16, name="junkG")
            ep_sb = aux_sb[0:4, EP_O:EP_O + 32]
            # h1 finishes first (last tile emits its h1 matmuls first), so
            # drive the whole h1 chain ahead of h0 on every engine
            REP_T = [psRep0, psRep1]
            for h in (1, 0):
                hs = slice(h * 512, (h + 1) * 512)
                st_, sp, _ = SSQ_T[h]
                nc.scalar.activation(
                    out=sq_s[0:4, hs], in_=st_[sp, :], func=Act.Sqrt)
            for h in (1, 0):
                hs = slice(h * 512, (h + 1) * 512)
                dt_, dp, _ = DOT_T[h]
                nc.vector.reciprocal_approx_fast(
                    out=r_s[0:4, hs], in_=sq_s[0:4, hs])
                nc.vector.tensor_mul(
                    out=sim_sb[:, hs], in0=dt_[dp, :], in1=r_s[0:4, hs])
            for h in (1, 0):
                hs = slice(h * 512, (h + 1) * 512)
                nc.tensor.matmul(
                    REP_T[h][96:128, :], lhsT=ep_sb, rhs=sim_sb[:, hs],
                    start=True, stop=True, skip_group_check=True,
                    tile_position=(0, 96))
            # hinge accumulate: h1 on ACT (relu+bias), h0 on DVE
            # ((simrep + ms) max 0, accumulated) -- the two run in parallel
            nc.scalar.activation(
                out=junkG[96:128, 512:1024], in_=psRep1[96:128, :],
                func=Act.Relu, bias=ms_sb[96:128, 0:1], scale=1.0,
                accum_out=out_sb[96:128, 1:2])
            nc.vector.scalar_tensor_tensor(
                out=junkG[96:128, 0:512],
                in0=psRep0[96:128, :], scalar=ms_sb[96:128, 0:1],
                in1=warm_sb[96:128, 0:512],
                op0=Alu.add, op1=Alu.max,
                accum_out=out_sb[96:128, 0:1])

            nc.sync.dma_start(out=outp[:, :], in_=out_sb[96:128, 0:3])

    nc.finalize()
    return nc


def _get_nc():
    if "nc" not in _CACHED:
        _CACHED["nc"] = _build_nc()
    return _CACHED["nc"]


def _host_prep(sent, query, pos_idx):
    """Build per-core input maps (all heavy prep is reshapes + bf16 cast)."""
    bf16 = ml_dtypes.bfloat16
    sent = np.ascontiguousarray(sent, dtype=np.float32)
    query = np.asarray(query, dtype=np.float32)
    pos_idx = np.asarray(pos_idx).astype(np.int64)

    qn = np.linalg.norm(query, axis=-1, keepdims=True)
    qhat = (query / np.maximum(qn, 1e-12)).astype(bf16)     # [B, D]

    # [B, 2, 128, L] bf16, d-chunk-major transposed tiles
    xt = sent.astype(bf16).transpose(0, 2, 1).reshape(B, 2, 128, L)

    in_maps = []
    for core in range(NCORES):
        ks = slice(core * BL, (core + 1) * BL)
        x2 = np.ascontiguousarray(
            xt[ks].transpose(2, 0, 1, 3).reshape(128, T * 1024))

        aux = np.zeros((128, AUXC), dtype=bf16)
        for c in range(2):
            for k in range(BL):
                aux[:, QWS_O + (c * 4 + k) * 4 + k] = qhat[core * BL + k,
                                                           c * 128:(c + 1) * 128]
        for k in range(BL):
            aux[:, OWS_O + k * 4 + k] = 1.0
        for k in range(BL):
            aux[k, EP_O + k * P:EP_O + (k + 1) * P] = 1.0
        for c in range(2):
            for k in range(BL):
                for j in range(P):
                    aux[:, QP_O + c * 32 + k * P + j] = qhat[
                        core * BL + k, c * 128:(c + 1) * 128]
                    aux[:, XP_O + c * 32 + k * P + j] = xt[
                        core * BL + k, c, :, pos_idx[core * BL + k, j]]
        aux[np.arange(64, 96), I32_O + np.arange(32)] = 1.0

        in_maps.append({"x2": x2, "aux": aux})
    return in_maps, pos_idx


def _host_finish(results, pos_idx):
    """Combine per-core (G[k,j], s_vec[k,j]) into the scalar loss."""
    g = np.zeros((B, P), dtype=np.float64)
    s = np.zeros((B, P), dtype=np.float64)
    for core, res in enumerate(results):
        o = res["outp"].astype(np.float64)          # [32, 3]
        g[core * BL:(core + 1) * BL] = (o[:, 0] + o[:, 1]).reshape(BL, P)
        s[core * BL:(core + 1) * BL] = o[:, 2].reshape(BL, P)

    loss = 0.0
    total = 0
    for b in range(B):
        _, first = np.unique(pos_idx[b], return_index=True)
        npos = len(first)
        total += npos * (L - npos)
        sb = s[b, first]
        loss += g[b, first].sum()
        loss -= np.maximum(sb[None, :] - sb[:, None] + MARGIN, 0.0).sum()
    return np.float32(loss / total)


def kernel(sent_embeddings, query_embeddings, pos_idx, splits=None, **_):
    import sys
    if "/opt/trn_rl_repo" not in sys.path:
        sys.path.insert(0, "/opt/trn_rl_repo")
    from concourse.bass_utils import run_bass_kernel_spmd

    in_maps, pos_idx = _host_prep(sent_embeddings, query_embeddings, pos_idx)
    nc = _get_nc()
    res = run_bass_kernel_spmd(nc, in_maps, core_ids=list(range(NCORES)))
    _CACHED["last_result"] = res
    return _host_finish(res.results, pos_idx)


if __name__ == "__main__":
    rng = np.random.default_rng(0)
    sent = rng.standard_normal((B, L, D), dtype=np.float32)
    query = rng.standard_normal((B, D), dtype=np.float32)
    pidx = np.stack([rng.choice(L, P, replace=False) for _ in range(B)])
    print(kernel(sent, query, pidx, L))


# revision 18
# speedup vs baseline: 1.0724x; 1.0522x over previous
"""Trainium2 Bass kernel for a contrastive hinge loss (bf16 rewrite).

Problem (B=32 splits, L=1024 candidates/split, P=8 positives/split, D=256):
    e = l2norm(sent), q = l2norm(query)
    sim[b,l] = e[b,l] . q[b]
    loss = sum_{b, p in pos_b, j in neg_b} relu(sim[b,j] - sim[b,p] + margin) / total

Strategy (data-parallel over B across 8 cores, 4 splits per core):
  Layout: D on partitions (2 chunks of 128), candidates on the free dim,
  everything bf16 on the wire (half the HBM traffic of fp32; PE runs
  1 col/cycle instead of fp32's multi-pass).

  Device per core:
    - x2 [128, 8*1024] bf16: tile t = (split k, d-chunk c) at cols t*1024.
      4 DMA chunks of 512KB, two HWDGE rings.
    - dot[k,l]: PE matmuls, lhsT = one-hot column matrix (col k = qhat_k
      chunk c), accumulating [4, 1024] in PSUM @p0-3 (col-group 0).
    - ssq[k,l]: squares on DVE/ACT/GpSimd, then PE matmuls with one-hot
      ones lhsT into PSUM @p32-35 (col-group 1, runs concurrent with dot).
    - sim = dot * rsqrt(ssq): ACT Sqrt (PSUM->SBUF) -> DVE reciprocal ->
      DVE mult (PSUM fp32 x SBUF fp32 -> bf16, cross-partition-base OK for
      mixed-space operands).
    - positives: host gathers the 32 positive columns; tiny PE matmuls
      (q.xP and Gram(xP)) @p64-95 + diagonal-mask STT give s_vec[32],
      ms = margin - s_vec.
    - hinge: replicate sim rows to 32 partitions via PE (lhsT = selector E),
      then ONE ACT Relu pass per column half with per-partition bias = ms
      and accum_out -> G[32].
  Host: normalizes queries, builds one-hot weights, gathers positives;
  finish: loss = [sum G - sum_{p,q in pos} relu(s_q - s_p + m)] / total.
"""

import numpy as np
import ml_dtypes

B, L, P, D = 32, 1024, 8, 256
NCORES = 8
BL = B // NCORES          # 4 splits per core
T = BL * 2                # 8 (split, chunk) tiles per core
MARGIN = 0.01
NWARM = 5                 # PE warm-up matmuls (HAM clock-gate)

# aux column layout (all bf16, [128, AUXC])
QWS_O = 0                 # 2c x 4k blocks of [128,4]: col k = qhat chunk c
OWS_O = 32                # 4k blocks of [128,4]: col k = ones
EP_O = 48                 # [4, 32] selector E[b, m] = (m//8 == b)
QP_O = 80                 # [128, 2*32]: col (c,k*8+j) = qhat_k chunk c
XP_O = 144                # [128, 2*32]: col (c,k*8+j) = x[k, pos_kj] chunk c
I32_O = 208               # [32,32] identity at partitions 64..95
AUXC = 240

# which engine squares tile t = k*2+c  (v=DVE, s=ACT, g=GpSimd)
SQ_ENG = ["g", "v", "s", "g", "v", "g", "s", "v"]

_CACHED = {}


def _build_nc():
    import concourse.bass as bass
    import concourse.mybir as mybir
    import concourse.tile as tile
    from concourse import bacc

    f32 = mybir.dt.float32
    bf16 = mybir.dt.bfloat16
    Alu = mybir.AluOpType
    Act = mybir.ActivationFunctionType

    nc = bacc.Bacc("TRN2")
    x2 = nc.dram_tensor("x2", [128, T * 1024], bf16, kind="ExternalInput")
    aux = nc.dram_tensor("aux", [128, AUXC], bf16, kind="ExternalInput")
    outp = nc.dram_tensor("outp", [32, 3], f32, kind="ExternalOutput")

    with tile.TileContext(nc) as tc:
        with (
            tc.tile_pool(name="sing", bufs=1) as sing,
            tc.tile_pool(name="pp", bufs=1, space="PSUM") as pp,
        ):
            aux_sb = sing.tile([128, AUXC], bf16, name="aux_sb")
            x_sb = sing.tile([128, T * 1024], bf16, name="x_sb")
            # aux first on the SP ring so the pos stage + qws unblock early
            nc.sync.dma_start(out=aux_sb[:, :], in_=aux[:, :])
            # x in 4 chunks of 2048 cols (= one split each) over the two
            # HWDGE rings; aggregate lands at the ~290 GB/s HBM limit
            for k in range(BL):
                eng = nc.scalar if k % 2 == 0 else nc.sync
                eng.dma_start(
                    out=x_sb[:, k * 2048:(k + 1) * 2048],
                    in_=x2[:, k * 2048:(k + 1) * 2048])

            # PSUM layout: one accumulation group per 2KB bank -- a start=True
            # matmul into a bank wipes any open accumulation there (verified
            # on HW), so dot/ssq/pos/rep each get private banks.
            # dot-h0 @[0:4] bank0 (col-grp 0), dot-h1 @[64:68] bank1 (grp 2),
            # ssq-h0 @[32:36] bank2 (grp 1), ssq-h1 @[96:100] bank3 (grp 3):
            # four concurrent rhs streams through the PE.
            # separate pp.tile per bank: Tile tracks PSUM deps per tile,
            # so readers of one region must not be chained to writers of
            # another (e.g. sqrt-h1 must not wait on the h0 matmuls)
            psDot0 = pp.tile([4, 512], f32, name="psDot0")
            psSsq0 = pp.tile([36, 512], f32, name="psSsq0")
            psDot1 = pp.tile([68, 512], f32, name="psDot1")
            psSsq1 = pp.tile([100, 512], f32, name="psSsq1")
            psRep0 = pp.tile([128, 512], f32, name="psRep0")
            psRep1 = pp.tile([128, 512], f32, name="psRep1")
            psPosA = pp.tile([96, 512], f32, name="psPosA")
            psPosB = pp.tile([96, 512], f32, name="psPosB")

            # ---- warmups (M=128 so the HAM activity monitor sees them) ----
            warm_sb = sing.tile([128, 512], bf16, name="warm_sb")
            nc.vector.memset(warm_sb[:, :], 0.0)
            for i in range(NWARM):
                nc.tensor.matmul(
                    psRep0[0:128, 0:512], lhsT=warm_sb[:, 0:128],
                    rhs=warm_sb[:, :],
                    start=True, stop=True, skip_group_check=True)

            # ---- positives (tiny, early; only needs aux) ----
            for c in range(2):
                qp = aux_sb[:, QP_O + c * 32:QP_O + (c + 1) * 32]
                xp = aux_sb[:, XP_O + c * 32:XP_O + (c + 1) * 32]
                nc.tensor.matmul(
                    psPosA[64:96, 0:32], lhsT=qp, rhs=xp,
                    start=(c == 0), stop=(c == 1), skip_group_check=True)
                nc.tensor.matmul(
                    psPosB[64:96, 0:32], lhsT=xp, rhs=xp,
                    start=(c == 0), stop=(c == 1), skip_group_check=True)
            pos_sb = sing.tile([96, 8], f32, name="pos_sb")
            junkP = sing.tile([96, 64], f32, name="junkP")
            i32_sb = aux_sb[64:96, I32_O:I32_O + 32]
            nc.vector.scalar_tensor_tensor(
                out=junkP[64:96, 0:32], in0=psPosA[64:96, 0:32], scalar=1.0,
                in1=i32_sb, op0=Alu.mult, op1=Alu.mult,
                accum_out=pos_sb[64:96, 0:1])
            nc.vector.scalar_tensor_tensor(
                out=junkP[64:96, 32:64], in0=psPosB[64:96, 0:32], scalar=1.0,
                in1=i32_sb, op0=Alu.mult, op1=Alu.mult,
                accum_out=pos_sb[64:96, 1:2])
            nc.scalar.activation(
                out=pos_sb[64:96, 2:3], in_=pos_sb[64:96, 1:2], func=Act.Sqrt)
            nc.vector.reciprocal(
                out=pos_sb[64:96, 3:4], in_=pos_sb[64:96, 2:3])
            nc.vector.tensor_mul(
                out=pos_sb[64:96, 4:5], in0=pos_sb[64:96, 0:1],
                in1=pos_sb[64:96, 3:4])
            out_sb = sing.tile([128, 3], f32, name="out_sb")
            ms_sb = sing.tile([128, 1], f32, name="ms_sb")
            # ms = margin - s_vec, moved to partitions 96..127 for the G pass
            nc.scalar.activation(
                out=ms_sb[96:128, 0:1], in_=pos_sb[64:96, 4:5],
                func=Act.Copy, bias=float(MARGIN), scale=-1.0)
            # s_vec to the output block (also cross-partition copy)
            nc.scalar.activation(
                out=out_sb[96:128, 2:3], in_=pos_sb[64:96, 4:5], func=Act.Copy)

            # ---- main: squares + dot/ssq matmuls per (split k, chunk c) ----
            # dot/ssq x h0/h1 go to four distinct PE column-groups, so the
            # four streams execute concurrently on the array.
            DOT_T = [(psDot0, slice(0, 4), (0, 0)),
                     (psDot1, slice(64, 68), (0, 64))]
            SSQ_T = [(psSsq0, slice(32, 36), (0, 32)),
                     (psSsq1, slice(96, 100), (0, 96))]
            xsq_sb = sing.tile([128, T * 1024], bf16, name="xsq_sb")
            for k in range(BL):
                for c in range(2):
                    t = k * 2 + c
                    seg = slice(t * 1024, (t + 1) * 1024)
                    e = SQ_ENG[t]
                    if e == "s":
                        nc.scalar.activation(
                            out=xsq_sb[:, seg], in_=x_sb[:, seg],
                            func=Act.Square)
                    elif e == "v":
                        nc.vector.tensor_mul(
                            out=xsq_sb[:, seg], in0=x_sb[:, seg],
                            in1=x_sb[:, seg])
                    else:
                        nc.gpsimd.tensor_mul(
                            out=xsq_sb[:, seg], in0=x_sb[:, seg],
                            in1=x_sb[:, seg])
                    qw = aux_sb[:, QWS_O + (c * 4 + k) * 4:
                                QWS_O + (c * 4 + k) * 4 + 4]
                    ow = aux_sb[:, OWS_O + k * 4:OWS_O + k * 4 + 4]
                    first = (k == 0 and c == 0)
                    last = (k == BL - 1 and c == 1)
                    horder = (1, 0) if last else (0, 1)
                    for h in horder:
                        xs = x_sb[:, t * 1024 + h * 512:t * 1024 + h * 512 + 512]
                        xq = xsq_sb[:, t * 1024 + h * 512:t * 1024 + h * 512 + 512]
                        dt_, dp, dtp = DOT_T[h]
                        st_, sp, stp = SSQ_T[h]
                        nc.tensor.matmul(
                            dt_[dp, :], lhsT=qw, rhs=xs,
                            start=first, stop=last, skip_group_check=True,
                            tile_position=dtp)
                        nc.tensor.matmul(
                            st_[sp, :], lhsT=ow, rhs=xq,
                            start=first, stop=last, skip_group_check=True,
                            tile_position=stp)

            # ---- finish: sim, replicate, hinge-accumulate ----
            # reciprocal_approx_fast requires partition base 0 (custom-DVE
            # uop breaks on sliced bases) -- keep the whole chain at p0-3
            sq_s = sing.tile([4, 1024], f32, name="sq_s")
            r_s = sing.tile([4, 1024], f32, name="r_s")
            sim_sb = sing.tile([4, 1024], bf16, name="sim_sb")
            junkG = sing.tile([128, 1024], bf16, name="junkG")
            ep_sb = aux_sb[0:4, EP_O:EP_O + 32]
            # h1 finishes first (last tile emits its h1 matmuls first), so
            # drive the whole h1 chain ahead of h0 on every engine
            REP_T = [psRep0, psRep1]
            for h in (1, 0):
                hs = slice(h * 512, (h + 1) * 512)
                st_, sp, _ = SSQ_T[h]
                nc.scalar.activation(
                    out=sq_s[0:4, hs], in_=st_[sp, :], func=Act.Sqrt)
            for h in (1, 0):
                hs = slice(h * 512, (h + 1) * 512)
                dt_, dp, _ = DOT_T[h]
                nc.vector.reciprocal_approx_fast(
                    out=r_s[0:4, hs], in_=sq_s[0:4, hs])
                nc.vector.tensor_mul(
                    out=sim_sb[:, hs], in0=dt_[dp, :], in1=r_s[0:4, hs])
            for h in (1, 0):
                hs = slice(h * 512, (h + 1) * 512)
                nc.tensor.matmul(
                    REP_T[h][96:128, :], lhsT=ep_sb, rhs=sim_sb[:, hs],
                    start=True, stop=True, skip_group_check=True,
                    tile_position=(0, 96))
            # hinge accumulate: h1 on ACT (relu+bias), h0 on DVE
            # ((simrep + ms) max 0, accumulated) -- the two run in parallel
            nc.scalar.activation(
                out=junkG[96:128, 512:1024], in_=psRep1[96:128, :],
                func=Act.Relu, bias=ms_sb[96:128, 0:1], scale=1.0,
                accum_out=out_sb[96:128, 1:2])
            nc.vector.scalar_tensor_tensor(
                out=junkG[96:128, 0:512],
                in0=psRep0[96:128, :], scalar=ms_sb[96:128, 0:1],
                in1=warm_sb[96:128, 0:512],
                op0=Alu.add, op1=Alu.max,
                accum_out=out_sb[96:128, 0:1])

            nc.sync.dma_start(out=outp[:, :], in_=out_sb[96:128, 0:3])

    nc.finalize()
    return nc


def _get_nc():
    if "nc" not in _CACHED:
        _CACHED["nc"] = _build_nc()
    return _CACHED["nc"]


def _host_prep(sent, query, pos_idx):
    """Build per-core input maps (all heavy prep is reshapes + bf16 cast)."""
    bf16 = ml_dtypes.bfloat16
    sent = np.ascontiguousarray(sent, dtype=np.float32)
    query = np.asarray(query, dtype=np.float32)
    pos_idx = np.asarray(pos_idx).astype(np.int64)

    qn = np.linalg.norm(query, axis=-1, keepdims=True)
    qhat = (query / np.maximum(qn, 1e-12)).astype(bf16)     # [B, D]

    # [B, 2, 128, L] bf16, d-chunk-major transposed tiles
    xt = sent.astype(bf16).transpose(0, 2, 1).reshape(B, 2, 128, L)

    in_maps = []
    for core in range(NCORES):
        ks = slice(core * BL, (core + 1) * BL)
        x2 = np.ascontiguousarray(
            xt[ks].transpose(2, 0, 1, 3).reshape(128, T * 1024))

        aux = np.zeros((128, AUXC), dtype=bf16)
        for c in range(2):
            for k in range(BL):
                aux[:, QWS_O + (c * 4 + k) * 4 + k] = qhat[core * BL + k,
                                                           c * 128:(c + 1) * 128]
        for k in range(BL):
            aux[:, OWS_O + k * 4 + k] = 1.0
        for k in range(BL):
            aux[k, EP_O + k * P:EP_O + (k + 1) * P] = 1.0
        for c in range(2):
            for k in range(BL):
                for j in range(P):
                    aux[:, QP_O + c * 32 + k * P + j] = qhat[
                        core * BL + k, c * 128:(c + 1) * 128]
                    aux[:, XP_O + c * 32 + k * P + j] = xt[
                        core * BL + k, c, :, pos_idx[core * BL + k, j]]
        aux[np.arange(64, 96), I32_O + np.arange(32)] = 1.0

        in_maps.append({"x2": x2, "aux": aux})
    return in_maps, pos_idx


def _host_finish(results, pos_idx):
    """Combine per-core (G[k,j], s_vec[k,j]) into the scalar loss."""
    g = np.zeros((B, P), dtype=np.float64)
    s = np.zeros((B, P), dtype=np.float64)
    for core, res in enumerate(results):
        o = res["outp"].astype(np.float64)          # [32, 3]
        g[core * BL:(core + 1) * BL] = (o[:, 0] + o[:, 1]).reshape(BL, P)
        s[core * BL:(core + 1) * BL] = o[:, 2].reshape(BL, P)

    loss = 0.0
    total = 0
    for b in range(B):
        _, first = np.unique(pos_idx[b], return_index=True)
        npos = len(first)
        total += npos * (L - npos)
        sb = s[b, first]
        loss += g[b, first].sum()
        loss -= np.maximum(sb[None, :] - sb[:, None] + MARGIN, 0.0).sum()
    return np.float32(loss / total)


def kernel(sent_embeddings, query_embeddings, pos_idx, splits=None, **_):
    import sys
    if "/opt/trn_rl_repo" not in sys.path:
        sys.path.insert(0, "/opt/trn_rl_repo")
    from concourse.bass_utils import run_bass_kernel_spmd

    in_maps, pos_idx = _host_prep(sent_embeddings, query_embeddings, pos_idx)
    nc = _get_nc()
    res = run_bass_kernel_spmd(nc, in_maps, core_ids=list(range(NCORES)))
    _CACHED["last_result"] = res
    return _host_finish(res.results, pos_idx)


if __name__ == "__main__":
    rng = np.random.default_rng(0)
    sent = rng.standard_normal((B, L, D), dtype=np.float32)
    query = rng.standard_normal((B, D), dtype=np.float32)
    pidx = np.stack([rng.choice(L, P, replace=False) for _ in range(B)])
    print(kernel(sent, query, pidx, L))
